# revision 13
# baseline (speedup 1.0000x reference)
"""Trainium2 Bass kernel for nn_BasicBlock (dense transformer block).

Sharding: data-parallel over batch — B=8 batch elements, one per NeuronCore,
zero collectives. Each core runs the full block on its [1024, 1024] slice.

Per-core structure (S=1024 tokens, D=1024, H=16 heads, d_k=64, d_ff=4096):
  - token-major residual stream [s-part, d-free]; PE transposes x and h1 into
    feature-major [d-part, s-free] for use as matmul contraction operands
  - qT/kT feature-major; v token-major augmented with a ones column so the
    attention BMM2 accumulates context rows 0..63 and the softmax denominator
    in row 64 of one PSUM group
  - causal attention computed as scoresT [s_k-part, s_q-free]: only column
    ranges right of the diagonal are computed (half the work); exp with fused
    1/sqrt(d_k) scale on ScalarE; strict lower-triangular mask applied to the
    single diagonal 128-block per (head, chunk)
  - denominator broadcast across 64 partitions via a K=1 PE matmul with a
    ones row; reciprocal+multiply normalizes ctx into concatT
  - all matmuls in float32r (fp32 bytes, TF32-class precision, full PE rate)
"""

import numpy as np
import concourse.bass as bass
import concourse.tile as tile
from concourse import bacc, mybir
from concourse.bass_utils import run_bass_kernel_spmd

F32 = mybir.dt.float32
F32R = mybir.dt.float32r
AF = mybir.ActivationFunctionType
OP = mybir.AluOpType

B, S, D, H, DK, DFF = 8, 1024, 1024, 16, 64, 4096
P = 128
DC = D // P       # 8 chunks of d_model
FC = DFF // P     # 32 chunks of d_ff
SC = S // P       # 8 chunks of sequence
EPS = 1e-5
DEN_EPS = 1e-30
SCALE = 0.125     # 1/sqrt(DK)


def _score_ranges(j):
    # per s_k chunk j: s_q column ranges right of the diagonal, cut at 512
    lo = P * j
    if lo < 512:
        return [(lo, 512), (512, 1024)]
    return [(lo, 1024)]


def _build():
    nc = bacc.Bacc("TRN2", target_bir_lowering=False, debug=False, num_devices=B)

    x_d = nc.dram_tensor("x", [S, D], F32, kind="ExternalInput").ap()
    wq_d = nc.dram_tensor("Wq", [D, D], F32, kind="ExternalInput").ap()
    wk_d = nc.dram_tensor("Wk", [D, D], F32, kind="ExternalInput").ap()
    wv_d = nc.dram_tensor("Wv", [D, D], F32, kind="ExternalInput").ap()
    wo_d = nc.dram_tensor("Wo", [D, D], F32, kind="ExternalInput").ap()
    w1_d = nc.dram_tensor("W1", [D, DFF], F32, kind="ExternalInput").ap()
    w2_d = nc.dram_tensor("W2", [DFF, D], F32, kind="ExternalInput").ap()
    bq_d = nc.dram_tensor("bq", [D], F32, kind="ExternalInput").ap()
    bk_d = nc.dram_tensor("bk", [D], F32, kind="ExternalInput").ap()
    bv_d = nc.dram_tensor("bv", [D], F32, kind="ExternalInput").ap()
    bo_d = nc.dram_tensor("bo", [D], F32, kind="ExternalInput").ap()
    b1_d = nc.dram_tensor("b1", [DFF], F32, kind="ExternalInput").ap()
    b2_d = nc.dram_tensor("b2", [D], F32, kind="ExternalInput").ap()
    g1_d = nc.dram_tensor("g1", [D], F32, kind="ExternalInput").ap()
    be1_d = nc.dram_tensor("beta1", [D], F32, kind="ExternalInput").ap()
    g3_d = nc.dram_tensor("g3", [D], F32, kind="ExternalInput").ap()
    be3_d = nc.dram_tensor("beta3", [D], F32, kind="ExternalInput").ap()
    id_d = nc.dram_tensor("ident", [P, P], F32, kind="ExternalInput").ap()
    mask_d = nc.dram_tensor("mask", [P, P], F32, kind="ExternalInput").ap()
    ones_d = nc.dram_tensor("ones", [P, P], F32, kind="ExternalInput").ap()
    out_d = nc.dram_tensor("out", [S, D], F32, kind="ExternalOutput").ap()

    def bcast_ap(dram_ap, n):
        return bass.AP(tensor=dram_ap.tensor, offset=dram_ap.offset,
                       ap=[[0, P], [1, n]])

    with tile.TileContext(nc) as tc:
      with tc.tile_pool(name="singles", bufs=1) as singles, \
           tc.tile_pool(name="sm", bufs=4) as sm:
        ident_sb = singles.tile([P, P], F32)
        mask_sb = singles.tile([P, P], F32)
        ones_sb = singles.tile([P, DK], F32R)
        eps_t = singles.tile([P, 1], F32)
        bq_sb = singles.tile([P, DC], F32)
        bk_sb = singles.tile([P, DC], F32)
        b1_sb = singles.tile([P, FC], F32)
        nc.sync.dma_start(ident_sb[:], id_d)
        nc.sync.dma_start(mask_sb[:], mask_d)
        nc.sync.dma_start(ones_sb[:], ones_d[:, 0:DK].bitcast(F32R))
        nc.vector.memset(eps_t[:], EPS)
        nc.sync.dma_start(bq_sb[:], bq_d.rearrange("(c p) -> p c", p=P))
        nc.sync.dma_start(bk_sb[:], bk_d.rearrange("(c p) -> p c", p=P))
        nc.sync.dma_start(b1_sb[:], b1_d.rearrange("(c p) -> p c", p=P))

        def ln_inplace(ap_1024, g_b, be_b):
            # layernorm over the 1024-wide free dim of ap_1024 [128, 1024]
            st = sm.tile([P, 2, 6], F32, tag="st", name="st")
            mv = sm.tile([P, 2], F32, tag="mv", name="mv")
            rs = sm.tile([P, 1], F32, tag="rs", name="rs")
            for g in range(2):
                nc.vector.bn_stats(st[:, g, :], ap_1024[:, 512 * g:512 * (g + 1)])
            nc.vector.bn_aggr(mv[:], st[:])
            nc.scalar.activation(rs[:], mv[:, 1:2], AF.Sqrt,
                                 bias=eps_t[:], scale=1.0)
            nc.vector.reciprocal(rs[:], rs[:])
            nc.vector.tensor_scalar(ap_1024, ap_1024, mv[:, 0:1], rs[:],
                                    op0=OP.subtract, op1=OP.mult)
            nc.vector.tensor_mul(ap_1024, ap_1024, g_b[:])
            nc.gpsimd.tensor_add(ap_1024, ap_1024, be_b[:])

        with tc.tile_pool(name="cat", bufs=1) as cat:
            concatT_sb = cat.tile([P, DC, S], F32R)

            # ======== phases 0/A/B: QKV + attention ========
            with tc.tile_pool(name="qkv", bufs=1) as qkv:
                qT_sb = qkv.tile([P, DC, S], F32R)
                kT_sb = qkv.tile([P, DC, S], F32R)
                vaug_sb = qkv.tile([P, SC, H, DK + 1], F32R)
                ones_bc = bass.AP(tensor=ones_d.tensor, offset=ones_d.offset,
                                  ap=[[0, P], [1, H]]).bitcast(F32R)
                for m in range(SC):
                    nc.sync.dma_start(vaug_sb[:, m, :, DK:DK + 1].squeeze(2),
                                      ones_bc)

                with tc.tile_pool(name="xTp", bufs=1) as xTp:
                    xT_sb = xTp.tile([P, DC, S], F32R)
                    # -------- phase 0: load x, PE-transpose to xT --------
                    with tc.tile_pool(name="x0", bufs=1) as x0p, \
                         tc.tile_pool(name="psT", bufs=4, space="PSUM") as psT:
                        x_sb = x0p.tile([P, SC, D], F32)
                        for m in range(SC):
                            nc.sync.dma_start(x_sb[:, m, :], x_d[P * m:P * (m + 1), :])
                        for m in range(SC):
                            for c in range(DC):
                                pt = psT.tile([P, P], F32, tag="pt", name="pt")
                                nc.tensor.transpose(
                                    pt[:], x_sb[:, m, P * c:P * (c + 1)], ident_sb[:])
                                nc.vector.tensor_copy(
                                    xT_sb[:, c, P * m:P * (m + 1)], pt[:])

                    # -------- phase A: QKV projections --------
                    with tc.tile_pool(name="wstr", bufs=9) as wstr, \
                         tc.tile_pool(name="bvb", bufs=1) as bvbp, \
                         tc.tile_pool(name="psA", bufs=4, space="PSUM") as psA:
                        bv_b = bvbp.tile([P, D], F32)
                        nc.gpsimd.dma_start(bv_b[:], bcast_ap(bv_d, D))

                        for (w_d, dst, bias) in ((wq_d, qT_sb, bq_sb),
                                                 (wk_d, kT_sb, bk_sb)):
                            wt = []
                            for k in range(DC):
                                t = wstr.tile([P, D], F32R, tag="w", name=f"w_{k}")
                                nc.sync.dma_start(
                                    t[:], w_d[P * k:P * (k + 1), :].bitcast(F32R))
                                wt.append(t)
                            for c in range(DC):
                                for n in range(2):
                                    cols = slice(512 * n, 512 * (n + 1))
                                    ps = psA.tile([P, 512], F32, tag="acc", name="acc")
                                    for k in range(DC):
                                        nc.tensor.matmul(
                                            ps[:], wt[k][:, P * c:P * (c + 1)],
                                            xT_sb[:, k, cols],
                                            start=(k == 0), stop=(k == DC - 1))
                                    nc.scalar.activation(
                                        dst[:, c, cols], ps[:], AF.Identity,
                                        bias=bias[:, c:c + 1], scale=1.0)
                        # V: token-major, into vaug (head-interleaved layout)
                        wt = []
                        for k in range(DC):
                            t = wstr.tile([P, D], F32R, tag="w", name=f"wv_{k}")
                            nc.sync.dma_start(
                                t[:], wv_d[P * k:P * (k + 1), :].bitcast(F32R))
                            wt.append(t)
                        for m in range(SC):
                            for n in range(2):
                                cols = slice(512 * n, 512 * (n + 1))
                                ps = psA.tile([P, 512], F32, tag="acc", name="acc")
                                for k in range(DC):
                                    nc.tensor.matmul(
                                        ps[:], xT_sb[:, k, P * m:P * (m + 1)],
                                        wt[k][:, cols],
                                        start=(k == 0), stop=(k == DC - 1))
                                nc.vector.tensor_add(
                                    vaug_sb[:, m, 8 * n:8 * (n + 1), 0:DK],
                                    ps[:].rearrange("p (h d) -> p h d", d=DK),
                                    bv_b[:, cols].rearrange("p (h d) -> p h d", d=DK))

                # -------- phase B: attention --------
                with tc.tile_pool(name="expp", bufs=1) as expp, \
                     tc.tile_pool(name="dsm", bufs=2) as dsm, \
                     tc.tile_pool(name="psS", bufs=2, space="PSUM") as psS, \
                     tc.tile_pool(name="psC", bufs=2, space="PSUM") as psC, \
                     tc.tile_pool(name="psB", bufs=2, space="PSUM") as psB:
                    for h in range(H):
                        ch = h // 2
                        off = DK * (h % 2)
                        expT = expp.tile([P, SC, S], F32R, tag="expT", name="expT")
                        for j in range(SC):
                            for (c0, c1) in _score_ranges(j):
                                ps = psS.tile([P, 512], F32, tag="sc", name="sc")
                                nc.tensor.matmul(
                                    ps[:, 0:c1 - c0],
                                    kT_sb[off:off + DK, ch, P * j:P * (j + 1)],
                                    qT_sb[off:off + DK, ch, c0:c1],
                                    start=True, stop=True)
                                nc.scalar.activation(
                                    expT[:, j, c0:c1], ps[:, 0:c1 - c0],
                                    AF.Exp, bias=0.0, scale=SCALE)
                            d0 = P * j
                            nc.vector.tensor_mul(expT[:, j, d0:d0 + P],
                                                 expT[:, j, d0:d0 + P].bitcast(F32),
                                                 mask_sb[:])
                        den = dsm.tile([P, S], F32R, tag="den", name="den")
                        rec = dsm.tile([P, S], F32, tag="rec", name="rec")
                        tmp = dsm.tile([P, S], F32R, tag="tmp", name="tmp")
                        for n in range(2):
                            cols = slice(512 * n, 512 * (n + 1))
                            psc = psC.tile([DK + 1, 512], F32, tag="ctx", name="ctx")
                            js = [j for j in range(SC) if P * j < 512 * (n + 1)]
                            for idx, j in enumerate(js):
                                s0 = max(512 * n, P * j)
                                nc.tensor.matmul(
                                    psc[:, s0 - 512 * n:512],
                                    vaug_sb[:, j, h, :],
                                    expT[:, j, s0:512 * (n + 1)],
                                    start=(idx == 0), stop=(idx == len(js) - 1),
                                    skip_group_check=True)
                            nc.scalar.activation(den[DK:DK + 1, cols],
                                                 psc[DK:DK + 1, :], AF.Copy,
                                                 bias=DEN_EPS, scale=1.0)
                            psb = psB.tile([DK, 512], F32, tag="bc", name="bc")
                            nc.tensor.matmul(psb[:], ones_sb[DK:DK + 1, :],
                                             den[DK:DK + 1, cols],
                                             start=True, stop=True)
                            nc.vector.reciprocal(rec[0:DK, cols], psb[:])
                            if off == 0:
                                nc.vector.tensor_mul(concatT_sb[0:DK, ch, cols],
                                                     psc[0:DK, :], rec[0:DK, cols])
                            else:
                                nc.vector.tensor_mul(tmp[0:DK, cols],
                                                     psc[0:DK, :], rec[0:DK, cols])
                        if off != 0:
                            nc.sync.dma_start(concatT_sb[DK:P, ch, :], tmp[0:DK, :])

            # ======== phases C/D/E under h1 ========
            with tc.tile_pool(name="h1p", bufs=1) as h1p:
                h1_sb = h1p.tile([P, SC, D], F32)

                # -------- phase C: out-proj + residual + LN1 --------
                with tc.tile_pool(name="wo", bufs=8) as wop, \
                     tc.tile_pool(name="x2", bufs=1) as x2p, \
                     tc.tile_pool(name="bcC", bufs=1) as bcC, \
                     tc.tile_pool(name="psA2", bufs=4, space="PSUM") as psA2:
                    bo_b = bcC.tile([P, D], F32)
                    g1_b = bcC.tile([P, D], F32)
                    be1_b = bcC.tile([P, D], F32)
                    nc.gpsimd.dma_start(bo_b[:], bcast_ap(bo_d, D))
                    nc.gpsimd.dma_start(g1_b[:], bcast_ap(g1_d, D))
                    nc.gpsimd.dma_start(be1_b[:], bcast_ap(be1_d, D))
                    x2_sb = x2p.tile([P, SC, D], F32)
                    for m in range(SC):
                        nc.sync.dma_start(x2_sb[:, m, :], x_d[P * m:P * (m + 1), :])
                    wt = []
                    for k in range(DC):
                        t = wop.tile([P, D], F32R, tag="wo", name=f"wo_{k}")
                        nc.sync.dma_start(
                            t[:], wo_d[P * k:P * (k + 1), :].bitcast(F32R))
                        wt.append(t)
                    for m in range(SC):
                        for n in range(2):
                            cols = slice(512 * n, 512 * (n + 1))
                            ps = psA2.tile([P, 512], F32, tag="acc2", name="acc2")
                            for k in range(DC):
                                nc.tensor.matmul(
                                    ps[:], concatT_sb[:, k, P * m:P * (m + 1)],
                                    wt[k][:, cols],
                                    start=(k == 0), stop=(k == DC - 1))
                            nc.vector.tensor_add(h1_sb[:, m, cols], ps[:],
                                                 x2_sb[:, m, cols])
                            nc.gpsimd.tensor_add(h1_sb[:, m, cols],
                                                 h1_sb[:, m, cols], bo_b[:, cols])
                        ln_inplace(h1_sb[:, m, :], g1_b, be1_b)

                # -------- phases D/E: transpose h1, FFN, LN2 --------
                with tc.tile_pool(name="h1Tp", bufs=1) as h1Tp:
                    h1T_sb = h1Tp.tile([P, DC, S], F32R)
                    with tc.tile_pool(name="psT2", bufs=4, space="PSUM") as psT2:
                        for m in range(SC):
                            for c in range(DC):
                                pt = psT2.tile([P, P], F32, tag="pt2", name="pt2")
                                nc.tensor.transpose(
                                    pt[:], h1_sb[:, m, P * c:P * (c + 1)], ident_sb[:])
                                nc.vector.tensor_copy(
                                    h1T_sb[:, c, P * m:P * (m + 1)], pt[:])

                    with tc.tile_pool(name="bcE", bufs=1) as bcE, \
                         tc.tile_pool(name="fT", bufs=1) as fTp, \
                         tc.tile_pool(name="outp", bufs=2) as outp:
                        b2_b = bcE.tile([P, D], F32)
                        g3_b = bcE.tile([P, D], F32)
                        be3_b = bcE.tile([P, D], F32)
                        nc.gpsimd.dma_start(b2_b[:], bcast_ap(b2_d, D))
                        nc.gpsimd.dma_start(g3_b[:], bcast_ap(g3_d, D))
                        nc.gpsimd.dma_start(be3_b[:], bcast_ap(be3_d, D))
                        w1_r = w1_d.rearrange("(k p) f -> p k f", p=P)
                        fT_sb = fTp.tile([P, FC, 512], F32R)
                        for hs in range(2):
                            scols = slice(512 * hs, 512 * (hs + 1))
                            with tc.tile_pool(name="w1s", bufs=4) as w1s, \
                                 tc.tile_pool(name="psF1", bufs=4,
                                              space="PSUM") as psF1:
                                for c in range(FC):
                                    w1t = w1s.tile([P, DC, P], F32R, tag="w1",
                                                   name=f"w1_{hs}_{c}")
                                    nc.sync.dma_start(
                                        w1t[:],
                                        w1_r[:, :, P * c:P * (c + 1)].bitcast(F32R))
                                    ps = psF1.tile([P, 512], F32, tag="f1", name="f1")
                                    for k in range(DC):
                                        nc.tensor.matmul(
                                            ps[:], w1t[:, k, :], h1T_sb[:, k, scols],
                                            start=(k == 0), stop=(k == DC - 1))
                                    nc.scalar.activation(
                                        fT_sb[:, c, :], ps[:], AF.Relu,
                                        bias=b1_sb[:, c:c + 1], scale=1.0)
                            with tc.tile_pool(name="w2s", bufs=3) as w2s, \
                                 tc.tile_pool(name="psF2", bufs=1,
                                              space="PSUM") as psF2:
                                pss = [psF2.tile([P, 512], F32, tag=f"f2_{i}",
                                                 name=f"f2_{hs}_{i}")
                                       for i in range(8)]
                                for k in range(FC):
                                    w2t = w2s.tile([P, D], F32R, tag="w2",
                                                   name=f"w2_{hs}_{k}")
                                    nc.sync.dma_start(
                                        w2t[:],
                                        w2_d[P * k:P * (k + 1), :].bitcast(F32R))
                                    for m4 in range(4):
                                        for n in range(2):
                                            nc.tensor.matmul(
                                                pss[2 * m4 + n][:],
                                                fT_sb[:, k, P * m4:P * (m4 + 1)],
                                                w2t[:, 512 * n:512 * (n + 1)],
                                                start=(k == 0), stop=(k == FC - 1))
                                for m4 in range(4):
                                    m = 4 * hs + m4
                                    o_t = outp.tile([P, D], F32, tag="ot", name="ot")
                                    for n in range(2):
                                        cols = slice(512 * n, 512 * (n + 1))
                                        nc.vector.tensor_add(
                                            o_t[:, cols], pss[2 * m4 + n][:],
                                            h1_sb[:, m, cols])
                                        nc.gpsimd.tensor_add(
                                            o_t[:, cols], o_t[:, cols], b2_b[:, cols])
                                    ln_inplace(o_t[:], g3_b, be3_b)
                                    nc.sync.dma_start(out_d[P * m:P * (m + 1), :],
                                                      o_t[:])

    nc.compile()
    return nc


_cached = None


def _get_prog():
    global _cached
    if _cached is None:
        _cached = _build()
    return _cached


def kernel(**inputs):
    x = np.asarray(inputs["x"], dtype=np.float32)
    assert x.shape == (B, S, D)
    ident = np.eye(P, dtype=np.float32)
    mask = np.triu(np.ones((P, P), dtype=np.float32), k=1)
    ones = np.ones((P, P), dtype=np.float32)
    common = {k: np.ascontiguousarray(np.asarray(inputs[k], dtype=np.float32))
              for k in ("Wq", "Wk", "Wv", "Wo", "W1", "W2", "bq", "bk", "bv",
                        "bo", "b1", "b2", "g1", "beta1", "g3", "beta3")}
    in_maps = [dict(common, x=np.ascontiguousarray(x[i]), ident=ident, mask=mask,
                    ones=ones)
               for i in range(B)]
    nc = _get_prog()
    res = run_bass_kernel_spmd(nc, in_maps, list(range(B)))
    return np.stack([res.results[i]["out"] for i in range(B)], axis=0)
